# revision 45
# baseline (speedup 1.0000x reference)
"""Multi-head attention (B=2, N=2048, C=768, H=12) on 8 Trainium2 NeuronCores.

Sharding: core c handles batch b=c//4 and head-group g=c%4 (3 heads, 192 dims).
Host side compacts rows where mask==0 out of x (fully-masked rows reduce to a
single mean-value projection row, computed on host), pre-transposes weight
slices, and casts matmul operands to bf16.

Device per core:
  q_T/k_T = W.T @ xcT                               (bf16 matmuls, fp32 psum)
  scores_T[k, q] = k_T-slice.T @ q_T                (keys on partitions)
  attn_T = exp(0.125*scores), two key tiles per activation instruction
     (pad keys have k == 0, so exp(0) = 1; they are cancelled by a zero in
      the v_aug sums column instead of an exp bias, which is what allows
      the two-tile exp fusion)
  outT[65, q] = sum_kt v_aug[kt].T @ attn_T[kt]     (row 64 = softmax sums)
  out_norm = outT[:64] * bcast(1/sums)              (DVE reads psum directly)
  proj[q, 768] = on_pair.T @ WoT_pair + on_solo.T @ [WoT_solo; bo/4]
     -- bias folded in as a 65th contraction row (on_solo row 64 = 1.0)
  bf16 ReduceScatter(add) in chunks over the 4 cores of the same batch;
  early chunks are issued as soon as their span's projection lands so the
  collective overlaps the rest of the kernel.

Schedule: score+exp units for the first two spans' pair-heads are emitted
block-interleaved with the QKV matmuls (o-accumulation deferred until v
lands), so the Act engine saturates on exps from ~5us while the PE grinds
through QKV; each span's projection is emitted one span behind attention.

Host reassembles kept rows and fills masked rows with the host-computed
mean-value projection row.
"""

import functools
import numpy as np
import ml_dtypes

import concourse.tile as tile
import concourse.mybir as mybir
from concourse import bacc
from concourse.bass_utils import run_bass_kernel_spmd

B, N, C = 2, 2048, 768
H, D = 12, 64
NCORES, NGROUPS, HPG = 8, 4, 3     # 4 head-groups of 3 heads; 2 batches
HD = HPG * D                       # 192 head dims per core
SCALE = float(D) ** -0.5           # 0.125
CT = C // 128                      # 6 contraction tiles of 128
BF16 = mybir.dt.bfloat16
F32 = mybir.dt.float32
NPBF16 = ml_dtypes.bfloat16

LAST_HW_NS = None
LAST_RESULT = None


def _spans(cnt):
    """Query spans, each <=512 wide (score psum tiles stay within one 2KB
    bank), multiple-of-4 boundaries (reused as ReduceScatter chunk edges).
    The last span is kept small (~128) so the end-of-kernel tail
    (attention + normalize + projection with nothing left to overlap)
    is as short as possible."""
    if cnt <= 512:
        return [(0, cnt)]
    tail = min(128, cnt - 4)
    nsp = -(-(cnt - tail) // 512)
    w = -(-(cnt - tail) // nsp // 4) * 4
    out = []
    s = 0
    while s < cnt - tail:
        out.append((s, min(w, cnt - tail - s)))
        s += w
    out.append((cnt - tail, tail))
    return out


def _rs_chunks(kp, cnt):
    """ReduceScatter row chunks: roughly one per span boundary, with each
    edge rounded down so every chunk's row count divides by 4 (the scatter
    group size); the last chunk absorbs the pad rows up to kp.  A chunk may
    start a few rows before its span boundary -- those rows were written by
    the previous span's projection, which completes before the chunk's
    collective is issued."""
    edges = sorted({(s // 4) * 4 for (s, _) in _spans(cnt)} | {kp})
    return [(edges[i], edges[i + 1]) for i in range(len(edges) - 1)]


@functools.lru_cache(maxsize=4)
def _build(kp, cnt, with_rs=True):
    """Build + compile the SPMD program for padded kept-count `kp`."""
    kt_n = kp // 128
    nc = bacc.Bacc("TRN2", target_bir_lowering=False, debug=False,
                   num_devices=NCORES)

    xcT = nc.dram_tensor("xcT", [C, kp], BF16, kind="ExternalInput").ap()
    vmk = nc.dram_tensor("vmk", [128, kt_n], BF16, kind="ExternalInput").ap()
    wqp = nc.dram_tensor("wqp", [C, 128], BF16, kind="ExternalInput").ap()
    wkp = nc.dram_tensor("wkp", [C, 128], BF16, kind="ExternalInput").ap()
    wkq = nc.dram_tensor("wkq", [C, 128], BF16, kind="ExternalInput").ap()
    wvT = nc.dram_tensor("wvT", [C, HD], BF16, kind="ExternalInput").ap()
    woT = nc.dram_tensor("woT", [HD + 1, C], BF16,
                         kind="ExternalInput").ap()   # row HD = bo/4

    out_ext = nc.dram_tensor("out", [kp // 4, C], BF16,
                             kind="ExternalOutput").ap()

    with tile.TileContext(nc) as tc:
        _emit(tc, nc, kp, cnt, kt_n,
              xcT, vmk, wqp, wkp, wkq, wvT, woT, out_ext, with_rs=with_rs)
    nc.compile()
    return nc


def _emit(tc, nc, kp, cnt, kt_n,
          xcT, vmk, wqp, wkp, wkq, wvT, woT, out_ext, with_rs=True):
    with tc.tile_pool(name="const", bufs=1) as consts, \
         tc.tile_pool(name="dram", bufs=1, space="DRAM") as dram:

        # ---- static loads ------------------------------------------------
        wk_sb = consts.tile([128, CT, 128], BF16)
        nc.gpsimd.dma_start(wk_sb[:], wkp.rearrange("(t p) d -> p t d", p=128))
        vmk_sb = consts.tile([128, kt_n], BF16)        # 1.0 = real key
        nc.sync.dma_start(vmk_sb[:], vmk[:])
        xcT_t = xcT.rearrange("(t p) n -> t p n", p=128)
        xq = consts.tile([128, CT, kp], BF16)          # x compact, transposed
        # block 0 of every contraction tile first (6 small contiguous DMAs,
        # ~2us total) so the first QKV matmuls -- and with them the first
        # exps -- start long before the full x transfer completes; the
        # remainder streams in as 6 per-tile continuation DMAs
        for s in range(0, kp, 512):
            w = min(512, kp - s)
            for ct in range(CT):
                nc.sync.dma_start(xq[:, ct, s:s + w], xcT_t[ct, :, s:s + w])
        wq_sb = consts.tile([128, CT, 128], BF16)
        nc.gpsimd.dma_start(wq_sb[:], wqp.rearrange("(t p) d -> p t d", p=128))
        wkq_sb = consts.tile([128, CT, 128], BF16)     # [k_solo | q_solo]
        nc.gpsimd.dma_start(wkq_sb[:], wkq.rearrange("(t p) d -> p t d", p=128))
        wv_sb = consts.tile([128, CT, HD], BF16)
        nc.gpsimd.dma_start(wv_sb[:], wvT.rearrange("(t p) d -> p t d", p=128))
        wo_pair = consts.tile([128, C], BF16)
        nc.gpsimd.dma_start(wo_pair[:], woT[0:128, :])
        wo_solo = consts.tile([65, C], BF16)           # rows 0:64 wo, 64 = bo/4
        nc.gpsimd.dma_start(wo_solo[0:64, :], woT[128:HD, :])
        nc.gpsimd.dma_start(wo_solo[64:65, :], woT[HD:HD + 1, :])

        rs_in = dram.tile([kp, C], BF16)
        rs_out = dram.tile([kp // 4, C], BF16)

        # ---- QKV / attention tiles --------------------------------------
        q_pair = consts.tile([128, kp], BF16, tag="q_pair")   # heads 0,1
        q_solo = consts.tile([64, kp], BF16, tag="q_solo")    # head 2
        k_pair = consts.tile([128, kp], BF16, tag="k_pair")
        v_aug = consts.tile([128, kt_n, HPG, D + 1], BF16, tag="v_aug")

        spans = _spans(cnt)

        # shared psum pools across QKV + attention + projection: no scope
        # barriers, slot rotation pipelines straight across phase changes
        with tc.tile_pool(name="ps", bufs=2, space="PSUM") as aps, \
             tc.tile_pool(name="o_ps", bufs=2, space="PSUM") as ops, \
             tc.tile_pool(name="pj_ps", bufs=2, space="PSUM") as jps, \
             tc.tile_pool(name="att_sb", bufs=40) as asb, \
             tc.tile_pool(name="nrm_sb", bufs=6) as nsb, \
             tc.tile_pool(name="on_sb", bufs=3) as onsb, \
             tc.tile_pool(name="pj_sb", bufs=6) as jsb:

            kq_pack = consts.tile([128, kp], BF16, tag="kq_pack")

            qkci = [0]

            def emit_w_block(w_sb, dst, s, w, qlim=None):
                """One 512-col block of a QKV weight-group matmul.  `qlim`
                trims the moving columns for q-side groups (queries past cnt
                are never read)."""
                if qlim is not None:
                    w = min(w, max(0, qlim - s))
                    if w == 0:
                        return
                ps = ops.tile([128, 512], F32, tag="o")
                for ct in range(CT):
                    nc.tensor.matmul(ps[:, :w], w_sb[:, ct, :],
                                     xq[:, ct, s:s + w],
                                     start=(ct == 0), stop=(ct == CT - 1))
                if qkci[0] < 2:
                    nc.scalar.copy(dst[:, s:s + w], ps[:, :w])
                else:
                    nc.vector.tensor_copy(dst[:, s:s + w], ps[:, :w])
                qkci[0] += 1

            def emit_v(kt):
                ps = ops.tile([128, 512], F32, tag="o")
                for ct in range(CT):
                    nc.tensor.matmul(ps[:, 0:HD],
                                     xq[:, ct, kt * 128:(kt + 1) * 128],
                                     wv_sb[:, ct, :],
                                     start=(ct == 0), stop=(ct == CT - 1))
                nc.vector.tensor_copy(
                    v_aug[:, kt, :, 0:D],
                    ps[:, 0:HD].rearrange("p (h d) -> p h d", h=HPG))

            blocks = [(s, min(512, kp - s)) for s in range(0, kp, 512)]

            # ---- attention + projection, span-major ----------------------
            span_on = []
            for (qs, qw) in spans:
                on_pair = onsb.tile([128, qw], BF16, tag="on_pair")
                on_solo = onsb.tile([65, qw], BF16, tag="on_solo")
                nc.vector.memset(on_solo[64:65, :qw], 1.0)    # bias row
                span_on.append((on_pair, on_solo))

            def head_src(h):
                if h < 2:
                    return k_pair, 64 * h, q_pair, 64 * h
                return kq_pack, 0, q_solo, 0

            def emit_s_pair(qs, qw, h, kt2):
                """Scores for key tiles kt2, kt2+1 into one 2-bank psum tile
                (one key tile per bank), so a single exp covers both."""
                k_src, k_lo, q_src, q_lo = head_src(h)
                n = min(2, kt_n - kt2)
                s_ps = aps.tile([128, 2, 512], F32, tag="s")
                for j in range(n):
                    kt = kt2 + j
                    nc.tensor.matmul(
                        s_ps[:, j, :qw],
                        k_src[k_lo:k_lo + D, kt * 128:(kt + 1) * 128],
                        q_src[q_lo:q_lo + D, qs:qs + qw],
                        start=True, stop=True)
                return s_ps, n

            def emit_exp(qw, s_ps, n):
                attn = asb.tile([128, 2, qw], BF16, tag="attn")
                nc.scalar.activation(attn[:, 0:n, :qw], s_ps[:, 0:n, :qw],
                                     mybir.ActivationFunctionType.Exp,
                                     scale=SCALE)
                return attn

            def emit_o(qs, qw, h, attns, on_pair, on_solo):
                """Accumulate attn @ v_aug over all key tiles, then normalize
                straight out of psum: out * bcast(1/sums), broadcast on the
                (otherwise idle) gpsimd engine."""
                o_ps = ops.tile([D + 1, qw], F32, tag="o")
                for (attn, kt2, n) in attns:
                    for j in range(n):
                        kt = kt2 + j
                        nc.tensor.matmul(o_ps[:, :qw], v_aug[:, kt, h, :],
                                         attn[:, j, :qw],
                                         start=(kt == 0),
                                         stop=(kt == kt_n - 1))
                rec = nsb.tile([1, qw], F32, tag="rec")
                nc.vector.reciprocal(rec[:, :qw], o_ps[D:D + 1, :qw])
                rec_bc = nsb.tile([D, qw], F32, tag="rec_bc")
                nc.gpsimd.partition_broadcast(rec_bc[:, :qw], rec[:, :qw])
                dst, dlo = (on_pair, 64 * h) if h < 2 else (on_solo, 0)
                nc.vector.tensor_mul(dst[dlo:dlo + D, :qw],
                                     o_ps[0:D, :qw], rec_bc[:, :qw])

            def emit_attn_head(qs, qw, h, on_pair, on_solo):
                """One (span, head): software-pipelined so the PE queue holds
                the next score pair before o(pair) and never stalls on exp."""
                o_ps = ops.tile([D + 1, qw], F32, tag="o")

                def exp_and_o(s_ps, n, kt2):
                    attn = emit_exp(qw, s_ps, n)
                    for j in range(n):
                        kt = kt2 + j
                        nc.tensor.matmul(o_ps[:, :qw], v_aug[:, kt, h, :],
                                         attn[:, j, :qw],
                                         start=(kt == 0),
                                         stop=(kt == kt_n - 1))

                prev = None
                for kt2 in range(0, kt_n, 2):
                    s_ps, n = emit_s_pair(qs, qw, h, kt2)
                    if prev is not None:
                        exp_and_o(*prev)
                    prev = (s_ps, n, kt2)
                exp_and_o(*prev)

                rec = nsb.tile([1, qw], F32, tag="rec")
                nc.vector.reciprocal(rec[:, :qw], o_ps[D:D + 1, :qw])
                rec_bc = nsb.tile([D, qw], F32, tag="rec_bc")
                nc.gpsimd.partition_broadcast(rec_bc[:, :qw], rec[:, :qw])
                dst, dlo = (on_pair, 64 * h) if h < 2 else (on_solo, 0)
                nc.vector.tensor_mul(dst[dlo:dlo + D, :qw],
                                     o_ps[0:D, :qw], rec_bc[:, :qw])

            def emit_proj(qs, qw, on_pair, on_solo, ci, mix=True,
                          part=None):
                """Output projection for one span; bias rides along as the
                65th contraction row of the solo matmul.  `part=(i, n)` emits
                only the i-th of n interleaved chunks of the qc loop."""
                qcs = list(range(0, qw, 128))
                if part is not None:
                    qcs = qcs[part[0]::part[1]]
                for qc in qcs:
                    cw = min(128, qw - qc)
                    for cs in range(0, C, 512):
                        ccw = min(512, C - cs)
                        pj = jps.tile([128, 512], F32, tag="pj")
                        nc.tensor.matmul(pj[:cw, :ccw],
                                         on_pair[:, qc:qc + cw],
                                         wo_pair[:, cs:cs + ccw],
                                         start=True, stop=False)
                        nc.tensor.matmul(pj[:cw, :ccw],
                                         on_solo[:, qc:qc + cw],
                                         wo_solo[:, cs:cs + ccw],
                                         start=False, stop=True)
                        pj_sb = jsb.tile([128, 512], BF16, tag="pj_sb")
                        if mix and ci % 2 == 0:
                            nc.scalar.copy(pj_sb[:cw, :ccw], pj[:cw, :ccw])
                        else:
                            nc.vector.tensor_copy(pj_sb[:cw, :ccw],
                                                  pj[:cw, :ccw])
                        ci += 1
                        nc.sync.dma_start(
                            rs_in[qs + qc:qs + qc + cw, cs:cs + ccw],
                            pj_sb[:cw, :ccw])
                return ci

            # Early phase: s+exp for heads 0/1 of the first two spans,
            # block-interleaved with QKV so the first exps start as soon as
            # the first k/q block lands, and the Act engine saturates while
            # the PE grinds through QKV.  o-accumulation is deferred until
            # v lands.
            early = [(si, h) for si in range(min(2, len(spans)))
                     for h in range(2)]
            npairs = -(-kt_n // 2)
            attns = {(si, h): [None] * npairs for (si, h) in early}

            def emit_unit(si, h, p):
                qs, qw = spans[si]
                kt2 = 2 * p
                s_ps, n = emit_s_pair(qs, qw, h, kt2)
                attns[si, h][p] = (emit_exp(qw, s_ps, n), kt2, n)

            # span-1 units keyed to k-block arrival
            for bi, (s, w) in enumerate(blocks):
                emit_w_block(wk_sb, k_pair, s, w)
                emit_w_block(wq_sb, q_pair, s, w, qlim=cnt)
                for p in range(npairs):
                    need = -(-min(2 * p + 2, kt_n) * 128 // 512) - 1
                    if need == bi:
                        for (si, h) in early[:2]:
                            emit_unit(si, h, p)

            # solo k/q group right away (span-1 head 2 needs it soon);
            # q_solo is un-packed per block so each small DMA only waits on
            # its own block's copy and never wedges the SP queue
            for (s, w) in blocks:
                emit_w_block(wkq_sb, kq_pack, s, w)
                nc.sync.dma_start(q_solo[:, s:s + w],
                                  kq_pack[64:128, s:s + w])

            def emit_vmask(h):          # sums column: 1.0 iff real key
                nc.vector.tensor_copy(v_aug[:, :, h, D:D + 1],
                                      vmk_sb.rearrange("p (t o) -> p t o",
                                                       o=1))

            # span-2 units woven with the remaining QKV work
            fillers = [(emit_vmask, (h,)) for h in range(HPG)]
            for kt in range(kt_n):
                fillers.append((emit_v, (kt,)))

            for (si, h) in early[2:]:
                for p in range(npairs):
                    emit_unit(si, h, p)
                    if fillers:
                        fn, args = fillers.pop(0)
                        fn(*args)
                    if fillers:
                        fn, args = fillers.pop(0)
                        fn(*args)
            for fn, args in fillers:
                fn(*args)

            ci = 0
            if len(spans) == 1:
                emit_o(*spans[0], 0, attns[0, 0], *span_on[0])
                emit_o(*spans[0], 1, attns[0, 1], *span_on[0])
                emit_attn_head(*spans[0], 2, *span_on[0])
            else:
                emit_o(*spans[0], 0, attns[0, 0], *span_on[0])
                emit_o(*spans[0], 1, attns[0, 1], *span_on[0])
                emit_attn_head(*spans[0], 2, *span_on[0])
                emit_o(*spans[1], 0, attns[1, 0], *span_on[1])
                emit_o(*spans[1], 1, attns[1, 1], *span_on[1])
                ci = emit_proj(*spans[0], *span_on[0], ci, mix=False)
                emit_attn_head(*spans[1], 2, *span_on[1])
                for si in range(2, len(spans)):
                    for h in range(HPG):
                        emit_attn_head(*spans[si], h, *span_on[si])
                        if h == 0:
                            ci = emit_proj(*spans[si - 1],
                                           *span_on[si - 1], ci)
            ci = emit_proj(*spans[-1], *span_on[-1], ci)

        # ---- chunked reduce-scatter over the 4 cores of this batch -------
        chunks = _rs_chunks(kp, cnt)
        if with_rs:
            o = 0
            for (r0, r1) in chunks:
                q = (r1 - r0) // 4
                nc.gpsimd.collective_compute(
                    "ReduceScatter", mybir.AluOpType.add,
                    replica_groups=[[0, 1, 2, 3], [4, 5, 6, 7]],
                    ins=[rs_in[r0:r1, :]],
                    outs=[rs_out[o:o + q, :]])
                nc.sync.dma_start(out_ext[o:o + q, :], rs_out[o:o + q, :])
                o += q
        else:
            nc.sync.dma_start(out_ext[:], rs_in[0:kp // 4, :])


def make_in_maps(inputs, kept, cnt, kp):
    x = np.asarray(inputs["x"], dtype=np.float32)
    Wq, Wk, Wv, Wo = (np.asarray(inputs[k], np.float32)
                      for k in ("Wq", "Wk", "Wv", "Wo"))
    bo = np.asarray(inputs["bo"], np.float32)
    woT_full = np.ascontiguousarray(Wo.T)          # [hd_in, c_out]
    in_maps = []
    for c in range(NCORES):
        b, g = divmod(c, NGROUPS)
        g0 = g * HD
        xc = np.zeros((kp, C), np.float32)
        xc[:cnt[b]] = x[b][kept[b]]
        vmk_flat = np.zeros(kp, np.float32)
        vmk_flat[:cnt[b]] = 1.0
        vmk = np.ascontiguousarray(vmk_flat.reshape(kp // 128, 128).T)
        wkq = np.concatenate([Wk[g0 + 128:g0 + HD],
                              Wq[g0 + 128:g0 + HD]], axis=0)
        woT = np.concatenate([woT_full[g0:g0 + HD],
                              (bo / NGROUPS).reshape(1, C)], axis=0)
        in_maps.append({
            "xcT": np.ascontiguousarray(xc.T).astype(NPBF16),
            "vmk": vmk.astype(NPBF16),
            "wqp": np.ascontiguousarray(Wq[g0:g0 + 128].T).astype(NPBF16),
            "wkp": np.ascontiguousarray(Wk[g0:g0 + 128].T).astype(NPBF16),
            "wkq": np.ascontiguousarray(wkq.T).astype(NPBF16),
            "wvT": np.ascontiguousarray(Wv[g0:g0 + HD].T).astype(NPBF16),
            "woT": np.ascontiguousarray(woT).astype(NPBF16),
        })
    return in_maps


def kernel(x, mask, Wq, Wk, Wv, Wo, bo):
    x = np.asarray(x, dtype=np.float32)
    mask = np.asarray(mask)
    Wq, Wk, Wv, Wo = (np.asarray(w, np.float32) for w in (Wq, Wk, Wv, Wo))
    bo = np.asarray(bo, np.float32)
    kept = [np.nonzero(mask[b])[0] for b in range(B)]
    cnt = [len(k) for k in kept]
    cnt_max = max(max(cnt), 1)
    kp = max(128, -(-cnt_max // 128) * 128)

    nc = _build(kp, cnt_max)
    in_maps = make_in_maps(
        {"x": x, "Wq": Wq, "Wk": Wk, "Wv": Wv, "Wo": Wo, "bo": bo},
        kept, cnt, kp)

    r = run_bass_kernel_spmd(nc, in_maps, core_ids=list(range(NCORES)))
    globals()["LAST_HW_NS"] = r.exec_time_ns or r.mean_exec_time_ns
    globals()["LAST_RESULT"] = r

    # masked rows: uniform attention over ALL rows -> host-computed row
    chunks = _rs_chunks(kp, cnt_max)
    out = np.empty((B, N, C), np.float32)
    for b in range(B):
        parts = [np.asarray(r.results[NGROUPS * b + i]["out"],
                            np.float32) for i in range(NGROUPS)]
        rows_full = np.empty((kp, C), np.float32)
        o = 0
        for (r0, r1) in chunks:
            q = (r1 - r0) // 4
            for i in range(NGROUPS):
                rows_full[r0 + i * q:r0 + (i + 1) * q] = parts[i][o:o + q]
            o += q
        out[b, kept[b]] = rows_full[:cnt[b]]
        mv = x[b].mean(0) @ Wv.T
        out[b, mask[b] == 0] = mv @ Wo.T + bo
    return out


# revision 54
# speedup vs baseline: 1.0357x; 1.0357x over previous
"""Multi-head attention (B=2, N=2048, C=768, H=12) on 8 Trainium2 NeuronCores.

Sharding: core c handles batch b=c//4 and head-group g=c%4 (3 heads, 192 dims).
Host side compacts rows where mask==0 out of x (fully-masked rows reduce to a
single mean-value projection row, computed on host), pre-transposes weight
slices, and casts matmul operands to bf16.

Device per core:
  q_T/k_T = W.T @ xcT                               (bf16 matmuls, fp32 psum)
  scores_T[k, q] = k_T-slice.T @ q_T                (keys on partitions)
  attn_T = exp(0.125*scores), two key tiles per activation instruction
     (pad keys have k == 0, so exp(0) = 1; they are cancelled by a zero in
      the v_aug sums column instead of an exp bias, which is what allows
      the two-tile exp fusion)
  outT[65, q] = sum_kt v_aug[kt].T @ attn_T[kt]     (row 64 = softmax sums)
  out_norm = outT[:64] * bcast(1/sums)              (DVE reads psum directly)
  proj[q, 768] = on_pair.T @ WoT_pair + on_solo.T @ [WoT_solo; bo/4]
     -- bias folded in as a 65th contraction row (on_solo row 64 = 1.0)
  bf16 ReduceScatter(add) in chunks over the 4 cores of the same batch;
  early chunks are issued as soon as their span's projection lands so the
  collective overlaps the rest of the kernel.

Schedule: score+exp units for the first two spans' pair-heads are emitted
block-interleaved with the QKV matmuls (o-accumulation deferred until v
lands), so the Act engine saturates on exps from ~5us while the PE grinds
through QKV; each span's projection is emitted one span behind attention.

Host reassembles kept rows and fills masked rows with the host-computed
mean-value projection row.
"""

import functools
import numpy as np
import ml_dtypes

import concourse.tile as tile
import concourse.mybir as mybir
from concourse import bacc
from concourse.bass_utils import run_bass_kernel_spmd

B, N, C = 2, 2048, 768
H, D = 12, 64
NCORES, NGROUPS, HPG = 8, 4, 3     # 4 head-groups of 3 heads; 2 batches
HD = HPG * D                       # 192 head dims per core
SCALE = float(D) ** -0.5           # 0.125
CT = C // 128                      # 6 contraction tiles of 128
BF16 = mybir.dt.bfloat16
F32 = mybir.dt.float32
NPBF16 = ml_dtypes.bfloat16

LAST_HW_NS = None
LAST_RESULT = None


def _spans(cnt):
    """Query spans, each <=512 wide (score psum tiles stay within one 2KB
    bank), multiple-of-4 boundaries (reused as ReduceScatter chunk edges).
    The last span is kept small (~128) so the end-of-kernel tail
    (attention + normalize + projection with nothing left to overlap)
    is as short as possible."""
    if cnt <= 512:
        return [(0, cnt)]
    tail = min(128, cnt - 4)
    nsp = -(-(cnt - tail) // 512)
    w = -(-(cnt - tail) // nsp // 4) * 4
    out = []
    s = 0
    while s < cnt - tail:
        out.append((s, min(w, cnt - tail - s)))
        s += w
    out.append((cnt - tail, tail))
    return out


def _rs_chunks(kp, cnt):
    """ReduceScatter row chunks: roughly one per span boundary, with each
    edge rounded down so every chunk's row count divides by 4 (the scatter
    group size); the last chunk absorbs the pad rows up to kp.  A chunk may
    start a few rows before its span boundary -- those rows were written by
    the previous span's projection, which completes before the chunk's
    collective is issued."""
    top = min(kp, -(-cnt // 4) * 4)    # don't reduce-scatter pure pad rows
    edges = sorted({(s // 4) * 4 for (s, _) in _spans(cnt)} | {top})
    return [(edges[i], edges[i + 1]) for i in range(len(edges) - 1)]


@functools.lru_cache(maxsize=4)
def _build(kp, cnt, with_rs=True):
    """Build + compile the SPMD program for padded kept-count `kp`."""
    kt_n = kp // 128
    nc = bacc.Bacc("TRN2", target_bir_lowering=False, debug=False,
                   num_devices=NCORES)

    xcT = nc.dram_tensor("xcT", [C, kp], BF16, kind="ExternalInput").ap()
    vmk = nc.dram_tensor("vmk", [128, kt_n], BF16, kind="ExternalInput").ap()
    wqp = nc.dram_tensor("wqp", [C, 128], BF16, kind="ExternalInput").ap()
    wkp = nc.dram_tensor("wkp", [C, 128], BF16, kind="ExternalInput").ap()
    wkq = nc.dram_tensor("wkq", [C, 128], BF16, kind="ExternalInput").ap()
    wvT = nc.dram_tensor("wvT", [C, HD], BF16, kind="ExternalInput").ap()
    woT = nc.dram_tensor("woT", [HD + 1, C], BF16,
                         kind="ExternalInput").ap()   # row HD = bo/4

    out_ext = nc.dram_tensor("out", [kp // 4, C], BF16,
                             kind="ExternalOutput").ap()

    with tile.TileContext(nc) as tc:
        _emit(tc, nc, kp, cnt, kt_n,
              xcT, vmk, wqp, wkp, wkq, wvT, woT, out_ext, with_rs=with_rs)
    nc.compile()
    return nc


def _emit(tc, nc, kp, cnt, kt_n,
          xcT, vmk, wqp, wkp, wkq, wvT, woT, out_ext, with_rs=True):
    with tc.tile_pool(name="const", bufs=1) as consts, \
         tc.tile_pool(name="dram", bufs=1, space="DRAM") as dram:

        # ---- static loads ------------------------------------------------
        wk_sb = consts.tile([128, CT, 128], BF16)
        nc.gpsimd.dma_start(wk_sb[:], wkp.rearrange("(t p) d -> p t d", p=128))
        vmk_sb = consts.tile([128, kt_n], BF16)        # 1.0 = real key
        nc.sync.dma_start(vmk_sb[:], vmk[:])
        xcT_t = xcT.rearrange("(t p) n -> t p n", p=128)
        xq = consts.tile([128, CT, kp], BF16)          # x compact, transposed
        # block 0 of every contraction tile first (6 small contiguous DMAs,
        # ~2us total) so the first QKV matmuls -- and with them the first
        # exps -- start long before the full x transfer completes; the
        # remainder streams in as 6 per-tile continuation DMAs
        for ct in range(CT):
            nc.sync.dma_start(xq[:, ct, 0:512], xcT_t[ct, :, 0:512])
        if cnt < kp:
            nc.vector.memset(xq[:, :, cnt:kp], 0.0)
        wq_sb = consts.tile([128, CT, 128], BF16)
        nc.gpsimd.dma_start(wq_sb[:], wqp.rearrange("(t p) d -> p t d", p=128))
        for ct in range(CT):
            w = min(kp, max(516, cnt)) - 512
            nc.gpsimd.dma_start(xq[:, ct, 512:512 + w],
                                xcT_t[ct, :, 512:512 + w])
        wkq_sb = consts.tile([128, CT, 128], BF16)     # [k_solo | q_solo]
        nc.gpsimd.dma_start(wkq_sb[:], wkq.rearrange("(t p) d -> p t d", p=128))
        wv_sb = consts.tile([128, CT, HD], BF16)
        nc.gpsimd.dma_start(wv_sb[:], wvT.rearrange("(t p) d -> p t d", p=128))
        wo_pair = consts.tile([128, C], BF16)
        nc.gpsimd.dma_start(wo_pair[:], woT[0:128, :])
        wo_solo = consts.tile([65, C], BF16)           # rows 0:64 wo, 64 = bo/4
        nc.gpsimd.dma_start(wo_solo[0:64, :], woT[128:HD, :])
        nc.gpsimd.dma_start(wo_solo[64:65, :], woT[HD:HD + 1, :])

        rs_in = dram.tile([kp, C], BF16)
        rs_out = dram.tile([kp // 4, C], BF16)
        chunks = _rs_chunks(kp, cnt)

        # ---- QKV / attention tiles --------------------------------------
        q_pair = consts.tile([128, kp], BF16, tag="q_pair")   # heads 0,1
        q_solo = consts.tile([64, kp], BF16, tag="q_solo")    # head 2
        k_pair = consts.tile([128, kp], BF16, tag="k_pair")
        v_aug = consts.tile([128, kt_n, HPG, D + 1], BF16, tag="v_aug")

        spans = _spans(cnt)

        # shared psum pools across QKV + attention + projection: no scope
        # barriers, slot rotation pipelines straight across phase changes
        with tc.tile_pool(name="ps", bufs=2, space="PSUM") as aps, \
             tc.tile_pool(name="o_ps", bufs=2, space="PSUM") as ops, \
             tc.tile_pool(name="pj_ps", bufs=2, space="PSUM") as jps, \
             tc.tile_pool(name="att_sb", bufs=40) as asb, \
             tc.tile_pool(name="nrm_sb", bufs=6) as nsb, \
             tc.tile_pool(name="on_sb", bufs=3) as onsb, \
             tc.tile_pool(name="pj_sb", bufs=6) as jsb:

            kq_pack = consts.tile([128, kp], BF16, tag="kq_pack")

            qkci = [0]

            def emit_w_block(w_sb, dst, s, w, qlim=None):
                """One 512-col block of a QKV weight-group matmul.  `qlim`
                trims the moving columns for q-side groups (queries past cnt
                are never read)."""
                if qlim is not None:
                    w = min(w, max(0, qlim - s))
                    if w == 0:
                        return
                ps = ops.tile([128, 512], F32, tag="o")
                for ct in range(CT):
                    nc.tensor.matmul(ps[:, :w], w_sb[:, ct, :],
                                     xq[:, ct, s:s + w],
                                     start=(ct == 0), stop=(ct == CT - 1))
                if qkci[0] < 2:
                    nc.scalar.copy(dst[:, s:s + w], ps[:, :w])
                else:
                    nc.vector.tensor_copy(dst[:, s:s + w], ps[:, :w])
                qkci[0] += 1

            def emit_v(kt):
                ps = ops.tile([128, 512], F32, tag="o")
                for ct in range(CT):
                    nc.tensor.matmul(ps[:, 0:HD],
                                     xq[:, ct, kt * 128:(kt + 1) * 128],
                                     wv_sb[:, ct, :],
                                     start=(ct == 0), stop=(ct == CT - 1))
                nc.vector.tensor_copy(
                    v_aug[:, kt, :, 0:D],
                    ps[:, 0:HD].rearrange("p (h d) -> p h d", h=HPG))

            warm = consts.tile([128, 512], BF16, tag="warm")
            nc.vector.memset(warm[:], 0.0)
            for _ in range(8):
                wps = jps.tile([128, 512], F32, tag="pj")
                nc.tensor.matmul(wps[:, :], warm[:, 0:128], warm[:, :],
                                 start=True, stop=True)

            blocks = [(s, min(512, kp - s)) for s in range(0, kp, 512)]

            # ---- attention + projection, span-major ----------------------
            span_on = []
            for (qs, qw) in spans:
                on_pair = onsb.tile([128, qw], BF16, tag="on_pair")
                on_solo = onsb.tile([65, qw], BF16, tag="on_solo")
                nc.vector.memset(on_solo[64:65, :qw], 1.0)    # bias row
                span_on.append((on_pair, on_solo))

            def head_src(h):
                if h < 2:
                    return k_pair, 64 * h, q_pair, 64 * h
                return kq_pack, 0, q_solo, 0

            def emit_s_pair(qs, qw, h, kt2):
                """Scores for key tiles kt2, kt2+1 into one 2-bank psum tile
                (one key tile per bank), so a single exp covers both."""
                k_src, k_lo, q_src, q_lo = head_src(h)
                n = min(2, kt_n - kt2)
                s_ps = aps.tile([128, 2, 512], F32, tag="s")
                for j in range(n):
                    kt = kt2 + j
                    nc.tensor.matmul(
                        s_ps[:, j, :qw],
                        k_src[k_lo:k_lo + D, kt * 128:(kt + 1) * 128],
                        q_src[q_lo:q_lo + D, qs:qs + qw],
                        start=True, stop=True)
                return s_ps, n

            def emit_exp(qw, s_ps, n):
                attn = asb.tile([128, 2, qw], BF16, tag="attn")
                nc.scalar.activation(attn[:, 0:n, :qw], s_ps[:, 0:n, :qw],
                                     mybir.ActivationFunctionType.Exp,
                                     scale=SCALE)
                return attn

            def emit_o(qs, qw, h, attns, on_pair, on_solo):
                """Accumulate attn @ v_aug over all key tiles, then normalize
                straight out of psum: out * bcast(1/sums), broadcast on the
                (otherwise idle) gpsimd engine."""
                o_ps = ops.tile([D + 1, qw], F32, tag="o")
                for (attn, kt2, n) in attns:
                    for j in range(n):
                        kt = kt2 + j
                        nc.tensor.matmul(o_ps[:, :qw], v_aug[:, kt, h, :],
                                         attn[:, j, :qw],
                                         start=(kt == 0),
                                         stop=(kt == kt_n - 1))
                rec = nsb.tile([1, qw], F32, tag="rec")
                nc.vector.reciprocal(rec[:, :qw], o_ps[D:D + 1, :qw])
                rec_bc = nsb.tile([D, qw], F32, tag="rec_bc")
                nc.gpsimd.partition_broadcast(rec_bc[:, :qw], rec[:, :qw])
                dst, dlo = (on_pair, 64 * h) if h < 2 else (on_solo, 0)
                nc.vector.tensor_mul(dst[dlo:dlo + D, :qw],
                                     o_ps[0:D, :qw], rec_bc[:, :qw])

            def emit_attn_head(qs, qw, h, on_pair, on_solo):
                """One (span, head): software-pipelined so the PE queue holds
                the next score pair before o(pair) and never stalls on exp."""
                o_ps = ops.tile([D + 1, qw], F32, tag="o")

                def exp_and_o(s_ps, n, kt2):
                    attn = emit_exp(qw, s_ps, n)
                    for j in range(n):
                        kt = kt2 + j
                        nc.tensor.matmul(o_ps[:, :qw], v_aug[:, kt, h, :],
                                         attn[:, j, :qw],
                                         start=(kt == 0),
                                         stop=(kt == kt_n - 1))

                prev = None
                for kt2 in range(0, kt_n, 2):
                    s_ps, n = emit_s_pair(qs, qw, h, kt2)
                    if prev is not None:
                        exp_and_o(*prev)
                    prev = (s_ps, n, kt2)
                exp_and_o(*prev)

                rec = nsb.tile([1, qw], F32, tag="rec")
                nc.vector.reciprocal(rec[:, :qw], o_ps[D:D + 1, :qw])
                rec_bc = nsb.tile([D, qw], F32, tag="rec_bc")
                nc.gpsimd.partition_broadcast(rec_bc[:, :qw], rec[:, :qw])
                dst, dlo = (on_pair, 64 * h) if h < 2 else (on_solo, 0)
                nc.vector.tensor_mul(dst[dlo:dlo + D, :qw],
                                     o_ps[0:D, :qw], rec_bc[:, :qw])

            def emit_proj(qs, qw, on_pair, on_solo, ci, mix=True,
                          part=None):
                """Output projection for one span; bias rides along as the
                65th contraction row of the solo matmul.  `part=(i, n)` emits
                only the i-th of n interleaved chunks of the qc loop."""
                qcs = list(range(0, qw, 128))
                if part is not None:
                    qcs = qcs[part[0]::part[1]]
                for qc in qcs:
                    cw = min(128, qw - qc)
                    for cs in range(0, C, 512):
                        ccw = min(512, C - cs)
                        pj = jps.tile([128, 512], F32, tag="pj")
                        nc.tensor.matmul(pj[:cw, :ccw],
                                         on_pair[:, qc:qc + cw],
                                         wo_pair[:, cs:cs + ccw],
                                         start=True, stop=False)
                        nc.tensor.matmul(pj[:cw, :ccw],
                                         on_solo[:, qc:qc + cw],
                                         wo_solo[:, cs:cs + ccw],
                                         start=False, stop=True)
                        pj_sb = jsb.tile([128, 512], BF16, tag="pj_sb")
                        if mix and ci % 2 == 0:
                            nc.scalar.copy(pj_sb[:cw, :ccw], pj[:cw, :ccw])
                        else:
                            nc.vector.tensor_copy(pj_sb[:cw, :ccw],
                                                  pj[:cw, :ccw])
                        ci += 1
                        nc.sync.dma_start(
                            rs_in[qs + qc:qs + qc + cw, cs:cs + ccw],
                            pj_sb[:cw, :ccw])
                return ci

            # Early phase: s+exp for heads 0/1 of the first two spans,
            # block-interleaved with QKV so the first exps start as soon as
            # the first k/q block lands, and the Act engine saturates while
            # the PE grinds through QKV.  o-accumulation is deferred until
            # v lands.
            early = [(si, h) for si in range(min(2, len(spans)))
                     for h in range(2)]
            npairs = -(-kt_n // 2)
            attns = {(si, h): [None] * npairs for (si, h) in early}

            def emit_unit(si, h, p):
                qs, qw = spans[si]
                kt2 = 2 * p
                s_ps, n = emit_s_pair(qs, qw, h, kt2)
                attns[si, h][p] = (emit_exp(qw, s_ps, n), kt2, n)

            # span-1 units keyed to k-block arrival
            for bi, (s, w) in enumerate(blocks):
                emit_w_block(wk_sb, k_pair, s, w)
                emit_w_block(wq_sb, q_pair, s, w, qlim=cnt)
                for p in range(npairs):
                    need = -(-min(2 * p + 2, kt_n) * 128 // 512) - 1
                    if need == bi:
                        for (si, h) in early[:2]:
                            emit_unit(si, h, p)

            # solo k/q group right away (span-1 head 2 needs it soon);
            # q_solo is un-packed per block so each small DMA only waits on
            # its own block's copy and never wedges the SP queue
            for (s, w) in blocks:
                emit_w_block(wkq_sb, kq_pack, s, w)
                nc.sync.dma_start(q_solo[:, s:s + w],
                                  kq_pack[64:128, s:s + w])

            def emit_vmask(h):          # sums column: 1.0 iff real key
                nc.vector.tensor_copy(v_aug[:, :, h, D:D + 1],
                                      vmk_sb.rearrange("p (t o) -> p t o",
                                                       o=1))

            # span-2 units woven with the remaining QKV work
            fillers = [(emit_vmask, (h,)) for h in range(HPG)]
            for kt in range(kt_n):
                fillers.append((emit_v, (kt,)))

            for (si, h) in early[2:]:
                for p in range(npairs):
                    emit_unit(si, h, p)
                    if fillers:
                        fn, args = fillers.pop(0)
                        fn(*args)
                    if fillers:
                        fn, args = fillers.pop(0)
                        fn(*args)
            for fn, args in fillers:
                fn(*args)

            ci = 0
            if len(spans) == 1:
                emit_o(*spans[0], 0, attns[0, 0], *span_on[0])
                emit_o(*spans[0], 1, attns[0, 1], *span_on[0])
                emit_attn_head(*spans[0], 2, *span_on[0])
            else:
                emit_o(*spans[0], 0, attns[0, 0], *span_on[0])
                emit_o(*spans[0], 1, attns[0, 1], *span_on[0])
                emit_attn_head(*spans[0], 2, *span_on[0])
                emit_o(*spans[1], 0, attns[1, 0], *span_on[1])
                emit_o(*spans[1], 1, attns[1, 1], *span_on[1])
                ci = emit_proj(*spans[0], *span_on[0], ci, mix=False)
                emit_attn_head(*spans[1], 2, *span_on[1])
                for si in range(2, len(spans)):
                    for h in range(HPG):
                        emit_attn_head(*spans[si], h, *span_on[si])
                        if h == 0:
                            ci = emit_proj(*spans[si - 1],
                                           *span_on[si - 1], ci)
            ci = emit_proj(*spans[-1], *span_on[-1], ci)

        # ---- chunked reduce-scatter over the 4 cores of this batch -------
        if with_rs:
            o = 0
            for (r0, r1) in chunks:
                q = (r1 - r0) // 4
                nc.gpsimd.collective_compute(
                    "ReduceScatter", mybir.AluOpType.add,
                    replica_groups=[[0, 1, 2, 3], [4, 5, 6, 7]],
                    ins=[rs_in[r0:r1, :]],
                    outs=[rs_out[o:o + q, :]])
                nc.sync.dma_start(out_ext[o:o + q, :], rs_out[o:o + q, :])
                o += q
        else:
            nc.sync.dma_start(out_ext[:], rs_in[0:out_ext.shape[0], :])


def make_in_maps(inputs, kept, cnt, kp):
    x = np.asarray(inputs["x"], dtype=np.float32)
    Wq, Wk, Wv, Wo = (np.asarray(inputs[k], np.float32)
                      for k in ("Wq", "Wk", "Wv", "Wo"))
    bo = np.asarray(inputs["bo"], np.float32)
    woT_full = np.ascontiguousarray(Wo.T)          # [hd_in, c_out]
    in_maps = []
    for c in range(NCORES):
        b, g = divmod(c, NGROUPS)
        g0 = g * HD
        xc = np.zeros((kp, C), np.float32)
        xc[:cnt[b]] = x[b][kept[b]]
        vmk_flat = np.zeros(kp, np.float32)
        vmk_flat[:cnt[b]] = 1.0
        vmk = np.ascontiguousarray(vmk_flat.reshape(kp // 128, 128).T)
        wkq = np.concatenate([Wk[g0 + 128:g0 + HD],
                              Wq[g0 + 128:g0 + HD]], axis=0)
        woT = np.concatenate([woT_full[g0:g0 + HD],
                              (bo / NGROUPS).reshape(1, C)], axis=0)
        in_maps.append({
            "xcT": np.ascontiguousarray(xc.T).astype(NPBF16),
            "vmk": vmk.astype(NPBF16),
            "wqp": np.ascontiguousarray(Wq[g0:g0 + 128].T).astype(NPBF16),
            "wkp": np.ascontiguousarray(Wk[g0:g0 + 128].T).astype(NPBF16),
            "wkq": np.ascontiguousarray(wkq.T).astype(NPBF16),
            "wvT": np.ascontiguousarray(Wv[g0:g0 + HD].T).astype(NPBF16),
            "woT": np.ascontiguousarray(woT).astype(NPBF16),
        })
    return in_maps


def kernel(x, mask, Wq, Wk, Wv, Wo, bo):
    x = np.asarray(x, dtype=np.float32)
    mask = np.asarray(mask)
    Wq, Wk, Wv, Wo = (np.asarray(w, np.float32) for w in (Wq, Wk, Wv, Wo))
    bo = np.asarray(bo, np.float32)
    kept = [np.nonzero(mask[b])[0] for b in range(B)]
    cnt = [len(k) for k in kept]
    cnt_max = max(max(cnt), 1)
    kp = max(128, -(-cnt_max // 128) * 128)

    nc = _build(kp, cnt_max)
    in_maps = make_in_maps(
        {"x": x, "Wq": Wq, "Wk": Wk, "Wv": Wv, "Wo": Wo, "bo": bo},
        kept, cnt, kp)

    r = run_bass_kernel_spmd(nc, in_maps, core_ids=list(range(NCORES)))
    globals()["LAST_HW_NS"] = r.exec_time_ns or r.mean_exec_time_ns
    globals()["LAST_RESULT"] = r

    # masked rows: uniform attention over ALL rows -> host-computed row
    chunks = _rs_chunks(kp, cnt_max)
    out = np.empty((B, N, C), np.float32)
    for b in range(B):
        parts = [np.asarray(r.results[NGROUPS * b + i]["out"],
                            np.float32) for i in range(NGROUPS)]
        rows_full = np.empty((kp, C), np.float32)
        o = 0
        for (r0, r1) in chunks:
            q = (r1 - r0) // 4
            for i in range(NGROUPS):
                rows_full[r0 + i * q:r0 + (i + 1) * q] = parts[i][o:o + q]
            o += q
        out[b, kept[b]] = rows_full[:cnt[b]]
        mv = x[b].mean(0) @ Wv.T
        out[b, mask[b] == 0] = mv @ Wo.T + bo
    return out


# revision 59
# speedup vs baseline: 1.0806x; 1.0434x over previous
"""Multi-head attention (B=2, N=2048, C=768, H=12) on 8 Trainium2 NeuronCores.

Sharding: core c handles batch b=c//4 and head-group g=c%4 (3 heads, 192 dims).
Host side compacts rows where mask==0 out of x (fully-masked rows reduce to a
single mean-value projection row, computed on host), pre-transposes weight
slices, and casts matmul operands to bf16.

Device per core:
  q_T/k_T = W.T @ xcT                               (bf16 matmuls, fp32 psum)
  scores_T[k, q] = k_T-slice.T @ q_T                (keys on partitions)
  attn_T = exp(0.125*scores), two key tiles per activation instruction
     (pad keys have k == 0, so exp(0) = 1; they are cancelled by a zero in
      the v_aug sums column instead of an exp bias, which is what allows
      the two-tile exp fusion)
  outT[65, q] = sum_kt v_aug[kt].T @ attn_T[kt]     (row 64 = softmax sums)
  out_norm = outT[:64] * bcast(1/sums)              (DVE reads psum directly)
  proj[q, 768] = on_pair.T @ WoT_pair + on_solo.T @ [WoT_solo; bo/4]
     -- bias folded in as a 65th contraction row (on_solo row 64 = 1.0)
  bf16 ReduceScatter(add) in chunks over the 4 cores of the same batch;
  early chunks are issued as soon as their span's projection lands so the
  collective overlaps the rest of the kernel.

Schedule: score+exp units for the first two spans' pair-heads are emitted
block-interleaved with the QKV matmuls (o-accumulation deferred until v
lands), so the Act engine saturates on exps from ~5us while the PE grinds
through QKV; each span's projection is emitted one span behind attention.

Host reassembles kept rows and fills masked rows with the host-computed
mean-value projection row.
"""

import functools
import numpy as np
import ml_dtypes

import concourse.tile as tile
import concourse.mybir as mybir
from concourse import bacc
from concourse.bass_utils import run_bass_kernel_spmd

B, N, C = 2, 2048, 768
H, D = 12, 64
NCORES, NGROUPS, HPG = 8, 4, 3     # 4 head-groups of 3 heads; 2 batches
HD = HPG * D                       # 192 head dims per core
SCALE = float(D) ** -0.5           # 0.125
CT = C // 128                      # 6 contraction tiles of 128
BF16 = mybir.dt.bfloat16
F32 = mybir.dt.float32
NPBF16 = ml_dtypes.bfloat16

LAST_HW_NS = None
LAST_RESULT = None


def _spans(cnt):
    """Query spans, each <=512 wide (score psum tiles stay within one 2KB
    bank), multiple-of-4 boundaries (reused as ReduceScatter chunk edges).
    The last span is kept small (~128) so the end-of-kernel tail
    (attention + normalize + projection with nothing left to overlap)
    is as short as possible."""
    if cnt <= 512:
        return [(0, cnt)]
    tail = min(128, cnt - 4)
    nsp = -(-(cnt - tail) // 512)
    w = -(-(cnt - tail) // nsp // 4) * 4
    out = []
    s = 0
    while s < cnt - tail:
        out.append((s, min(w, cnt - tail - s)))
        s += w
    out.append((cnt - tail, tail))
    return out


def _rs_chunks(kp, cnt):
    """ReduceScatter row chunks: roughly one per span boundary, with each
    edge rounded down so every chunk's row count divides by 4 (the scatter
    group size); the last chunk absorbs the pad rows up to kp.  A chunk may
    start a few rows before its span boundary -- those rows were written by
    the previous span's projection, which completes before the chunk's
    collective is issued."""
    top = min(kp, -(-cnt // 4) * 4)    # don't reduce-scatter pure pad rows
    edges = sorted({(s // 4) * 4 for (s, _) in _spans(cnt)} | {top})
    return [(edges[i], edges[i + 1]) for i in range(len(edges) - 1)]


@functools.lru_cache(maxsize=4)
def _build(kp, cnt, with_rs=True):
    """Build + compile the SPMD program for padded kept-count `kp`."""
    kt_n = kp // 128
    nc = bacc.Bacc("TRN2", target_bir_lowering=False, debug=False,
                   num_devices=NCORES)

    xcT = nc.dram_tensor("xcT", [C, kp], BF16, kind="ExternalInput").ap()
    vmk = nc.dram_tensor("vmk", [128, kt_n], BF16, kind="ExternalInput").ap()
    wqp = nc.dram_tensor("wqp", [C, 128], BF16, kind="ExternalInput").ap()
    wkp = nc.dram_tensor("wkp", [C, 128], BF16, kind="ExternalInput").ap()
    wkq = nc.dram_tensor("wkq", [C, 128], BF16, kind="ExternalInput").ap()
    wvT = nc.dram_tensor("wvT", [C, HD], BF16, kind="ExternalInput").ap()
    woT = nc.dram_tensor("woT", [HD + 1, C], BF16,
                         kind="ExternalInput").ap()   # row HD = bo/4

    out_ext = nc.dram_tensor("out", [kp // 4, C], BF16,
                             kind="ExternalOutput").ap()

    with tile.TileContext(nc) as tc:
        _emit(tc, nc, kp, cnt, kt_n,
              xcT, vmk, wqp, wkp, wkq, wvT, woT, out_ext, with_rs=with_rs)
    nc.compile()
    return nc


def _emit(tc, nc, kp, cnt, kt_n,
          xcT, vmk, wqp, wkp, wkq, wvT, woT, out_ext, with_rs=True):
    with tc.tile_pool(name="const", bufs=1) as consts, \
         tc.tile_pool(name="dram", bufs=1, space="DRAM") as dram:

        # ---- static loads ------------------------------------------------
        wk_sb = consts.tile([128, CT, 128], BF16)
        nc.gpsimd.dma_start(wk_sb[:], wkp.rearrange("(t p) d -> p t d", p=128))
        vmk_sb = consts.tile([128, kt_n], BF16)        # 1.0 = real key
        nc.sync.dma_start(vmk_sb[:], vmk[:])
        xcT_t = xcT.rearrange("(t p) n -> t p n", p=128)
        xq = consts.tile([128, CT, kp], BF16)          # x compact, transposed
        # block 0 of every contraction tile first (6 small contiguous DMAs,
        # ~2us total) so the first QKV matmuls -- and with them the first
        # exps -- start long before the full x transfer completes; the
        # remainder streams in as 6 per-tile continuation DMAs
        for ct in range(CT):
            nc.sync.dma_start(xq[:, ct, 0:512], xcT_t[ct, :, 0:512])
        if cnt < kp:
            nc.vector.memset(xq[:, :, cnt:kp], 0.0)
        wq_sb = consts.tile([128, CT, 128], BF16)
        nc.gpsimd.dma_start(wq_sb[:], wqp.rearrange("(t p) d -> p t d", p=128))
        for ct in range(CT):
            w = min(kp, max(516, cnt)) - 512
            q = nc.gpsimd if ct % 2 else nc.sync
            q.dma_start(xq[:, ct, 512:512 + w], xcT_t[ct, :, 512:512 + w])
        wkq_sb = consts.tile([128, CT, 128], BF16)     # [k_solo | q_solo]
        nc.gpsimd.dma_start(wkq_sb[:], wkq.rearrange("(t p) d -> p t d", p=128))
        wv_sb = consts.tile([128, CT, HD], BF16)
        nc.gpsimd.dma_start(wv_sb[:], wvT.rearrange("(t p) d -> p t d", p=128))
        wo_pair = consts.tile([128, C], BF16)
        nc.gpsimd.dma_start(wo_pair[:], woT[0:128, :])
        wo_solo = consts.tile([65, C], BF16)           # rows 0:64 wo, 64 = bo/4
        nc.gpsimd.dma_start(wo_solo[0:64, :], woT[128:HD, :])
        nc.gpsimd.dma_start(wo_solo[64:65, :], woT[HD:HD + 1, :])

        rs_in = dram.tile([kp, C], BF16)
        rs_out = dram.tile([kp // 4, C], BF16)
        chunks = _rs_chunks(kp, cnt)

        # ---- QKV / attention tiles --------------------------------------
        q_pair = consts.tile([128, kp], BF16, tag="q_pair")   # heads 0,1
        q_solo = consts.tile([64, kp], BF16, tag="q_solo")    # head 2
        k_pair = consts.tile([128, kp], BF16, tag="k_pair")
        v_aug = consts.tile([128, kt_n, HPG, D + 1], BF16, tag="v_aug")

        spans = _spans(cnt)

        # shared psum pools across QKV + attention + projection: no scope
        # barriers, slot rotation pipelines straight across phase changes
        with tc.tile_pool(name="ps", bufs=2, space="PSUM") as aps, \
             tc.tile_pool(name="o_ps", bufs=2, space="PSUM") as ops, \
             tc.tile_pool(name="pj_ps", bufs=2, space="PSUM") as jps, \
             tc.tile_pool(name="att_sb", bufs=40) as asb, \
             tc.tile_pool(name="nrm_sb", bufs=6) as nsb, \
             tc.tile_pool(name="on_sb", bufs=3) as onsb, \
             tc.tile_pool(name="pj_sb", bufs=6) as jsb:

            kq_pack = consts.tile([128, kp], BF16, tag="kq_pack")

            qkci = [0]

            def emit_w_block(w_sb, dst, s, w, qlim=None):
                """One 512-col block of a QKV weight-group matmul.  `qlim`
                trims the moving columns for q-side groups (queries past cnt
                are never read)."""
                if qlim is not None:
                    w = min(w, max(0, qlim - s))
                    if w == 0:
                        return
                ps = ops.tile([128, 512], F32, tag="o")
                for ct in range(CT):
                    nc.tensor.matmul(ps[:, :w], w_sb[:, ct, :],
                                     xq[:, ct, s:s + w],
                                     start=(ct == 0), stop=(ct == CT - 1))
                if qkci[0] < 2:
                    nc.scalar.copy(dst[:, s:s + w], ps[:, :w])
                else:
                    nc.vector.tensor_copy(dst[:, s:s + w], ps[:, :w])
                qkci[0] += 1

            def emit_v(kt):
                ps = ops.tile([128, 512], F32, tag="o")
                for ct in range(CT):
                    nc.tensor.matmul(ps[:, 0:HD],
                                     xq[:, ct, kt * 128:(kt + 1) * 128],
                                     wv_sb[:, ct, :],
                                     start=(ct == 0), stop=(ct == CT - 1))
                nc.vector.tensor_copy(
                    v_aug[:, kt, :, 0:D],
                    ps[:, 0:HD].rearrange("p (h d) -> p h d", h=HPG))

            warm = consts.tile([128, 512], BF16, tag="warm")
            nc.vector.memset(warm[:], 0.0)
            for _ in range(8):
                wps = jps.tile([128, 512], F32, tag="pj")
                nc.tensor.matmul(wps[:, :], warm[:, 0:128], warm[:, :],
                                 start=True, stop=True)

            blocks = [(s, min(512, kp - s)) for s in range(0, kp, 512)]

            # ---- attention + projection, span-major ----------------------
            span_on = []
            for (qs, qw) in spans:
                on_pair = onsb.tile([128, qw], BF16, tag="on_pair")
                on_solo = onsb.tile([65, qw], BF16, tag="on_solo")
                nc.vector.memset(on_solo[64:65, :qw], 1.0)    # bias row
                span_on.append((on_pair, on_solo))

            def head_src(h):
                if h < 2:
                    return k_pair, 64 * h, q_pair, 64 * h
                return kq_pack, 0, q_solo, 0

            def emit_s_pair(qs, qw, h, kt2):
                """Scores for key tiles kt2, kt2+1 into one 2-bank psum tile
                (one key tile per bank), so a single exp covers both."""
                k_src, k_lo, q_src, q_lo = head_src(h)
                n = min(2, kt_n - kt2)
                s_ps = aps.tile([128, 2, 512], F32, tag="s")
                for j in range(n):
                    kt = kt2 + j
                    nc.tensor.matmul(
                        s_ps[:, j, :qw],
                        k_src[k_lo:k_lo + D, kt * 128:(kt + 1) * 128],
                        q_src[q_lo:q_lo + D, qs:qs + qw],
                        start=True, stop=True)
                return s_ps, n

            def emit_exp(qw, s_ps, n):
                attn = asb.tile([128, 2, qw], BF16, tag="attn")
                nc.scalar.activation(attn[:, 0:n, :qw], s_ps[:, 0:n, :qw],
                                     mybir.ActivationFunctionType.Exp,
                                     scale=SCALE)
                return attn

            def emit_o(qs, qw, h, attns, on_pair, on_solo):
                """Accumulate attn @ v_aug over all key tiles, then normalize
                straight out of psum: out * bcast(1/sums), broadcast on the
                (otherwise idle) gpsimd engine."""
                o_ps = ops.tile([D + 1, qw], F32, tag="o")
                for (attn, kt2, n) in attns:
                    for j in range(n):
                        kt = kt2 + j
                        nc.tensor.matmul(o_ps[:, :qw], v_aug[:, kt, h, :],
                                         attn[:, j, :qw],
                                         start=(kt == 0),
                                         stop=(kt == kt_n - 1))
                rec = nsb.tile([1, qw], F32, tag="rec")
                nc.vector.reciprocal(rec[:, :qw], o_ps[D:D + 1, :qw])
                rec_bc = nsb.tile([D, qw], F32, tag="rec_bc")
                nc.gpsimd.partition_broadcast(rec_bc[:, :qw], rec[:, :qw])
                dst, dlo = (on_pair, 64 * h) if h < 2 else (on_solo, 0)
                nc.vector.tensor_mul(dst[dlo:dlo + D, :qw],
                                     o_ps[0:D, :qw], rec_bc[:, :qw])

            def emit_attn_head(qs, qw, h, on_pair, on_solo):
                """One (span, head): software-pipelined so the PE queue holds
                the next score pair before o(pair) and never stalls on exp."""
                o_ps = ops.tile([D + 1, qw], F32, tag="o")

                def exp_and_o(s_ps, n, kt2):
                    attn = emit_exp(qw, s_ps, n)
                    for j in range(n):
                        kt = kt2 + j
                        nc.tensor.matmul(o_ps[:, :qw], v_aug[:, kt, h, :],
                                         attn[:, j, :qw],
                                         start=(kt == 0),
                                         stop=(kt == kt_n - 1))

                prev = None
                for kt2 in range(0, kt_n, 2):
                    s_ps, n = emit_s_pair(qs, qw, h, kt2)
                    if prev is not None:
                        exp_and_o(*prev)
                    prev = (s_ps, n, kt2)
                exp_and_o(*prev)

                rec = nsb.tile([1, qw], F32, tag="rec")
                nc.vector.reciprocal(rec[:, :qw], o_ps[D:D + 1, :qw])
                rec_bc = nsb.tile([D, qw], F32, tag="rec_bc")
                nc.gpsimd.partition_broadcast(rec_bc[:, :qw], rec[:, :qw])
                dst, dlo = (on_pair, 64 * h) if h < 2 else (on_solo, 0)
                nc.vector.tensor_mul(dst[dlo:dlo + D, :qw],
                                     o_ps[0:D, :qw], rec_bc[:, :qw])

            def emit_proj(qs, qw, on_pair, on_solo, ci, mix=True,
                          part=None):
                """Output projection for one span; bias rides along as the
                65th contraction row of the solo matmul.  `part=(i, n)` emits
                only the i-th of n interleaved chunks of the qc loop."""
                qcs = list(range(0, qw, 128))
                if part is not None:
                    qcs = qcs[part[0]::part[1]]
                for qc in qcs:
                    cw = min(128, qw - qc)
                    for cs in range(0, C, 512):
                        ccw = min(512, C - cs)
                        pj = jps.tile([128, 512], F32, tag="pj")
                        nc.tensor.matmul(pj[:cw, :ccw],
                                         on_pair[:, qc:qc + cw],
                                         wo_pair[:, cs:cs + ccw],
                                         start=True, stop=False)
                        nc.tensor.matmul(pj[:cw, :ccw],
                                         on_solo[:, qc:qc + cw],
                                         wo_solo[:, cs:cs + ccw],
                                         start=False, stop=True)
                        pj_sb = jsb.tile([128, 512], BF16, tag="pj_sb")
                        if mix and ci % 2 == 0:
                            nc.scalar.copy(pj_sb[:cw, :ccw], pj[:cw, :ccw])
                        else:
                            nc.vector.tensor_copy(pj_sb[:cw, :ccw],
                                                  pj[:cw, :ccw])
                        ci += 1
                        nc.sync.dma_start(
                            rs_in[qs + qc:qs + qc + cw, cs:cs + ccw],
                            pj_sb[:cw, :ccw])
                return ci

            # Early phase: s+exp for heads 0/1 of the first two spans,
            # block-interleaved with QKV so the first exps start as soon as
            # the first k/q block lands, and the Act engine saturates while
            # the PE grinds through QKV.  o-accumulation is deferred until
            # v lands.
            early = [(si, h) for si in range(min(2, len(spans)))
                     for h in range(2)]
            npairs = -(-kt_n // 2)
            attns = {(si, h): [None] * npairs for (si, h) in early}

            def emit_unit(si, h, p):
                qs, qw = spans[si]
                kt2 = 2 * p
                s_ps, n = emit_s_pair(qs, qw, h, kt2)
                attns[si, h][p] = (emit_exp(qw, s_ps, n), kt2, n)

            # span-1 units keyed to k-block arrival
            for bi, (s, w) in enumerate(blocks):
                emit_w_block(wk_sb, k_pair, s, w)
                emit_w_block(wq_sb, q_pair, s, w, qlim=cnt)
                for p in range(npairs):
                    need = -(-min(2 * p + 2, kt_n) * 128 // 512) - 1
                    if need == bi:
                        for (si, h) in early[:2]:
                            emit_unit(si, h, p)

            # solo k/q group right away (span-1 head 2 needs it soon);
            # q_solo is un-packed per block so each small DMA only waits on
            # its own block's copy and never wedges the SP queue
            for (s, w) in blocks:
                emit_w_block(wkq_sb, kq_pack, s, w)
                nc.sync.dma_start(q_solo[:, s:s + w],
                                  kq_pack[64:128, s:s + w])

            def emit_vmask(h):          # sums column: 1.0 iff real key
                nc.vector.tensor_copy(v_aug[:, :, h, D:D + 1],
                                      vmk_sb.rearrange("p (t o) -> p t o",
                                                       o=1))

            # span-2 units woven with the remaining QKV work
            fillers = [(emit_vmask, (h,)) for h in range(HPG)]
            for kt in range(kt_n):
                fillers.append((emit_v, (kt,)))

            for (si, h) in early[2:]:
                for p in range(npairs):
                    emit_unit(si, h, p)
                    if fillers:
                        fn, args = fillers.pop(0)
                        fn(*args)
                    if fillers:
                        fn, args = fillers.pop(0)
                        fn(*args)
            for fn, args in fillers:
                fn(*args)

            def emit_units(si, h):
                attns[si, h] = []
                for p in range(npairs):
                    qs, qw = spans[si]
                    s_ps, n = emit_s_pair(qs, qw, h, 2 * p)
                    attns[si, h].append((emit_exp(qw, s_ps, n), 2 * p, n))

            ci = 0
            if len(spans) == 1:
                emit_units(0, 2)
                emit_o(*spans[0], 0, attns[0, 0], *span_on[0])
                emit_o(*spans[0], 1, attns[0, 1], *span_on[0])
                emit_o(*spans[0], 2, attns[0, 2], *span_on[0])
            else:
                emit_units(0, 2)
                emit_o(*spans[0], 0, attns[0, 0], *span_on[0])
                emit_o(*spans[0], 1, attns[0, 1], *span_on[0])
                emit_o(*spans[0], 2, attns[0, 2], *span_on[0])
                emit_units(1, 2)
                emit_o(*spans[1], 0, attns[1, 0], *span_on[1])
                emit_o(*spans[1], 1, attns[1, 1], *span_on[1])
                ci = emit_proj(*spans[0], *span_on[0], ci, mix=False)
                emit_o(*spans[1], 2, attns[1, 2], *span_on[1])
                for si in range(2, len(spans)):
                    for h in range(HPG):
                        emit_units(si, h)
                    emit_o(*spans[si], 0, attns[si, 0], *span_on[si])
                    ci = emit_proj(*spans[si - 1], *span_on[si - 1], ci)
                    emit_o(*spans[si], 1, attns[si, 1], *span_on[si])
                    emit_o(*spans[si], 2, attns[si, 2], *span_on[si])
            ci = emit_proj(*spans[-1], *span_on[-1], ci)

        # ---- chunked reduce-scatter over the 4 cores of this batch -------
        if with_rs:
            o = 0
            for (r0, r1) in chunks:
                q = (r1 - r0) // 4
                nc.gpsimd.collective_compute(
                    "ReduceScatter", mybir.AluOpType.add,
                    replica_groups=[[0, 1, 2, 3], [4, 5, 6, 7]],
                    ins=[rs_in[r0:r1, :]],
                    outs=[rs_out[o:o + q, :]])
                nc.sync.dma_start(out_ext[o:o + q, :], rs_out[o:o + q, :])
                o += q
        else:
            nc.sync.dma_start(out_ext[:], rs_in[0:out_ext.shape[0], :])


def make_in_maps(inputs, kept, cnt, kp):
    x = np.asarray(inputs["x"], dtype=np.float32)
    Wq, Wk, Wv, Wo = (np.asarray(inputs[k], np.float32)
                      for k in ("Wq", "Wk", "Wv", "Wo"))
    bo = np.asarray(inputs["bo"], np.float32)
    woT_full = np.ascontiguousarray(Wo.T)          # [hd_in, c_out]
    in_maps = []
    for c in range(NCORES):
        b, g = divmod(c, NGROUPS)
        g0 = g * HD
        xc = np.zeros((kp, C), np.float32)
        xc[:cnt[b]] = x[b][kept[b]]
        vmk_flat = np.zeros(kp, np.float32)
        vmk_flat[:cnt[b]] = 1.0
        vmk = np.ascontiguousarray(vmk_flat.reshape(kp // 128, 128).T)
        wkq = np.concatenate([Wk[g0 + 128:g0 + HD],
                              Wq[g0 + 128:g0 + HD]], axis=0)
        woT = np.concatenate([woT_full[g0:g0 + HD],
                              (bo / NGROUPS).reshape(1, C)], axis=0)
        in_maps.append({
            "xcT": np.ascontiguousarray(xc.T).astype(NPBF16),
            "vmk": vmk.astype(NPBF16),
            "wqp": np.ascontiguousarray(Wq[g0:g0 + 128].T).astype(NPBF16),
            "wkp": np.ascontiguousarray(Wk[g0:g0 + 128].T).astype(NPBF16),
            "wkq": np.ascontiguousarray(wkq.T).astype(NPBF16),
            "wvT": np.ascontiguousarray(Wv[g0:g0 + HD].T).astype(NPBF16),
            "woT": np.ascontiguousarray(woT).astype(NPBF16),
        })
    return in_maps


def kernel(x, mask, Wq, Wk, Wv, Wo, bo):
    x = np.asarray(x, dtype=np.float32)
    mask = np.asarray(mask)
    Wq, Wk, Wv, Wo = (np.asarray(w, np.float32) for w in (Wq, Wk, Wv, Wo))
    bo = np.asarray(bo, np.float32)
    kept = [np.nonzero(mask[b])[0] for b in range(B)]
    cnt = [len(k) for k in kept]
    cnt_max = max(max(cnt), 1)
    kp = max(128, -(-cnt_max // 128) * 128)

    nc = _build(kp, cnt_max)
    in_maps = make_in_maps(
        {"x": x, "Wq": Wq, "Wk": Wk, "Wv": Wv, "Wo": Wo, "bo": bo},
        kept, cnt, kp)

    r = run_bass_kernel_spmd(nc, in_maps, core_ids=list(range(NCORES)))
    globals()["LAST_HW_NS"] = r.exec_time_ns or r.mean_exec_time_ns
    globals()["LAST_RESULT"] = r

    # masked rows: uniform attention over ALL rows -> host-computed row
    chunks = _rs_chunks(kp, cnt_max)
    out = np.empty((B, N, C), np.float32)
    for b in range(B):
        parts = [np.asarray(r.results[NGROUPS * b + i]["out"],
                            np.float32) for i in range(NGROUPS)]
        rows_full = np.empty((kp, C), np.float32)
        o = 0
        for (r0, r1) in chunks:
            q = (r1 - r0) // 4
            for i in range(NGROUPS):
                rows_full[r0 + i * q:r0 + (i + 1) * q] = parts[i][o:o + q]
            o += q
        out[b, kept[b]] = rows_full[:cnt[b]]
        mv = x[b].mean(0) @ Wv.T
        out[b, mask[b] == 0] = mv @ Wo.T + bo
    return out


# revision 66
# speedup vs baseline: 1.0820x; 1.0012x over previous
"""Multi-head attention (B=2, N=2048, C=768, H=12) on 8 Trainium2 NeuronCores.

Sharding: core c handles batch b=c//4 and head-group g=c%4 (3 heads, 192 dims).
Host side compacts rows where mask==0 out of x (fully-masked rows reduce to a
single mean-value projection row, computed on host), pre-transposes weight
slices, and casts matmul operands to bf16.

Device per core:
  q_T/k_T = W.T @ xcT                               (bf16 matmuls, fp32 psum)
  scores_T[k, q] = k_T-slice.T @ q_T                (keys on partitions)
  attn_T = exp(0.125*scores), two key tiles per activation instruction
     (pad keys have k == 0, so exp(0) = 1; they are cancelled by a zero in
      the v_aug sums column instead of an exp bias, which is what allows
      the two-tile exp fusion)
  outT[65, q] = sum_kt v_aug[kt].T @ attn_T[kt]     (row 64 = softmax sums)
  out_norm = outT[:64] * bcast(1/sums)              (DVE reads psum directly)
  proj[q, 768] = on_pair.T @ WoT_pair + on_solo.T @ [WoT_solo; bo/4]
     -- bias folded in as a 65th contraction row (on_solo row 64 = 1.0)
  bf16 ReduceScatter(add) in chunks over the 4 cores of the same batch;
  early chunks are issued as soon as their span's projection lands so the
  collective overlaps the rest of the kernel.

Schedule: score+exp units for the first two spans' pair-heads are emitted
block-interleaved with the QKV matmuls (o-accumulation deferred until v
lands), so the Act engine saturates on exps from ~5us while the PE grinds
through QKV; each span's projection is emitted one span behind attention.

Host reassembles kept rows and fills masked rows with the host-computed
mean-value projection row.
"""

import functools
import numpy as np
import ml_dtypes

import concourse.tile as tile
import concourse.mybir as mybir
from concourse import bacc
from concourse.bass_utils import run_bass_kernel_spmd

B, N, C = 2, 2048, 768
H, D = 12, 64
NCORES, NGROUPS, HPG = 8, 4, 3     # 4 head-groups of 3 heads; 2 batches
HD = HPG * D                       # 192 head dims per core
SCALE = float(D) ** -0.5           # 0.125
CT = C // 128                      # 6 contraction tiles of 128
BF16 = mybir.dt.bfloat16
F32 = mybir.dt.float32
NPBF16 = ml_dtypes.bfloat16

LAST_HW_NS = None
LAST_RESULT = None


def _spans(cnt):
    """Query spans, each <=512 wide (score psum tiles stay within one 2KB
    bank), multiple-of-4 boundaries (reused as ReduceScatter chunk edges).
    The last span is kept small (~128) so the end-of-kernel tail
    (attention + normalize + projection with nothing left to overlap)
    is as short as possible."""
    if cnt <= 512:
        return [(0, cnt)]
    tail = min(128, cnt - 4)
    nsp = -(-(cnt - tail) // 512)
    w = -(-(cnt - tail) // nsp // 4) * 4
    out = []
    s = 0
    while s < cnt - tail:
        out.append((s, min(w, cnt - tail - s)))
        s += w
    out.append((cnt - tail, tail))
    return out


def _rs_chunks(kp, cnt):
    """ReduceScatter row chunks: roughly one per span boundary, with each
    edge rounded down so every chunk's row count divides by 4 (the scatter
    group size); the last chunk absorbs the pad rows up to kp.  A chunk may
    start a few rows before its span boundary -- those rows were written by
    the previous span's projection, which completes before the chunk's
    collective is issued."""
    top = min(kp, -(-cnt // 4) * 4)    # don't reduce-scatter pure pad rows
    edges = sorted({(s // 4) * 4 for (s, _) in _spans(cnt)} | {top})
    return [(edges[i], edges[i + 1]) for i in range(len(edges) - 1)]


@functools.lru_cache(maxsize=4)
def _build(kp, cnt, with_rs=True):
    """Build + compile the SPMD program for padded kept-count `kp`."""
    kt_n = kp // 128
    nc = bacc.Bacc("TRN2", target_bir_lowering=False, debug=False,
                   num_devices=NCORES)

    xcT = nc.dram_tensor("xcT", [C, kp], BF16, kind="ExternalInput").ap()
    vmk = nc.dram_tensor("vmk", [128, kt_n], BF16, kind="ExternalInput").ap()
    wqp = nc.dram_tensor("wqp", [C, 128], BF16, kind="ExternalInput").ap()
    wkp = nc.dram_tensor("wkp", [C, 128], BF16, kind="ExternalInput").ap()
    wkq = nc.dram_tensor("wkq", [C, 128], BF16, kind="ExternalInput").ap()
    wvT = nc.dram_tensor("wvT", [C, HD], BF16, kind="ExternalInput").ap()
    woT = nc.dram_tensor("woT", [HD + 1, C], BF16,
                         kind="ExternalInput").ap()   # row HD = bo/4

    out_ext = nc.dram_tensor("out", [kp // 4, C], BF16,
                             kind="ExternalOutput").ap()

    with tile.TileContext(nc) as tc:
        _emit(tc, nc, kp, cnt, kt_n,
              xcT, vmk, wqp, wkp, wkq, wvT, woT, out_ext, with_rs=with_rs)
    nc.compile()
    return nc


def _emit(tc, nc, kp, cnt, kt_n,
          xcT, vmk, wqp, wkp, wkq, wvT, woT, out_ext, with_rs=True):
    with tc.tile_pool(name="const", bufs=1) as consts, \
         tc.tile_pool(name="dram", bufs=1, space="DRAM") as dram:

        # ---- static loads ------------------------------------------------
        wk_sb = consts.tile([128, CT, 128], BF16)
        nc.gpsimd.dma_start(wk_sb[:], wkp.rearrange("(t p) d -> p t d", p=128))
        vmk_sb = consts.tile([128, kt_n], BF16)        # 1.0 = real key
        nc.sync.dma_start(vmk_sb[:], vmk[:])
        xcT_t = xcT.rearrange("(t p) n -> t p n", p=128)
        xq = consts.tile([128, CT, kp], BF16)          # x compact, transposed
        # block 0 of every contraction tile first (6 small contiguous DMAs,
        # ~2us total) so the first QKV matmuls -- and with them the first
        # exps -- start long before the full x transfer completes; the
        # remainder streams in as 6 per-tile continuation DMAs
        for ct in range(CT):
            nc.sync.dma_start(xq[:, ct, 0:512], xcT_t[ct, :, 0:512])
        if cnt < kp:
            nc.vector.memset(xq[:, :, cnt:kp], 0.0)
        wq_sb = consts.tile([128, CT, 128], BF16)
        nc.gpsimd.dma_start(wq_sb[:], wqp.rearrange("(t p) d -> p t d", p=128))
        for ct in range(CT):
            w = min(kp, max(516, cnt)) - 512
            q = nc.gpsimd if ct % 2 else nc.sync
            q.dma_start(xq[:, ct, 512:512 + w], xcT_t[ct, :, 512:512 + w])
        wkq_sb = consts.tile([128, CT, 128], BF16)     # [k_solo | q_solo]
        nc.gpsimd.dma_start(wkq_sb[:], wkq.rearrange("(t p) d -> p t d", p=128))
        wv_sb = consts.tile([128, CT, HD], BF16)
        nc.gpsimd.dma_start(wv_sb[:], wvT.rearrange("(t p) d -> p t d", p=128))
        wo_pair = consts.tile([128, C], BF16)
        nc.gpsimd.dma_start(wo_pair[:], woT[0:128, :])
        wo_solo = consts.tile([65, C], BF16)           # rows 0:64 wo, 64 = bo/4
        nc.gpsimd.dma_start(wo_solo[0:64, :], woT[128:HD, :])
        nc.gpsimd.dma_start(wo_solo[64:65, :], woT[HD:HD + 1, :])

        rs_in = dram.tile([kp, C], BF16)
        rs_out = dram.tile([kp // 4, C], BF16)
        chunks = _rs_chunks(kp, cnt)

        # ---- QKV / attention tiles --------------------------------------
        q_pair = consts.tile([128, kp], BF16, tag="q_pair")   # heads 0,1
        q_solo = consts.tile([64, kp], BF16, tag="q_solo")    # head 2
        k_pair = consts.tile([128, kp], BF16, tag="k_pair")
        v_aug = consts.tile([128, kt_n, HPG, D + 1], BF16, tag="v_aug")

        spans = _spans(cnt)

        # shared psum pools across QKV + attention + projection: no scope
        # barriers, slot rotation pipelines straight across phase changes
        with tc.tile_pool(name="ps", bufs=2, space="PSUM") as aps, \
             tc.tile_pool(name="o_ps", bufs=3, space="PSUM") as ops, \
             tc.tile_pool(name="pj_ps", bufs=1, space="PSUM") as jps, \
             tc.tile_pool(name="att_sb", bufs=40) as asb, \
             tc.tile_pool(name="nrm_sb", bufs=6) as nsb, \
             tc.tile_pool(name="on_sb", bufs=3) as onsb, \
             tc.tile_pool(name="pj_sb", bufs=6) as jsb:

            kq_pack = consts.tile([128, kp], BF16, tag="kq_pack")

            qkci = [0]

            def emit_w_block(w_sb, dst, s, w, qlim=None):
                """One 512-col block of a QKV weight-group matmul.  `qlim`
                trims the moving columns for q-side groups (queries past cnt
                are never read)."""
                if qlim is not None:
                    w = min(w, max(0, qlim - s))
                    if w == 0:
                        return
                ps = ops.tile([128, 512], F32, tag="o")
                for ct in range(CT):
                    nc.tensor.matmul(ps[:, :w], w_sb[:, ct, :],
                                     xq[:, ct, s:s + w],
                                     start=(ct == 0), stop=(ct == CT - 1))
                if qkci[0] < 2:
                    nc.scalar.copy(dst[:, s:s + w], ps[:, :w])
                else:
                    nc.vector.tensor_copy(dst[:, s:s + w], ps[:, :w])
                qkci[0] += 1

            def emit_v(kt):
                ps = ops.tile([128, 512], F32, tag="o")
                for ct in range(CT):
                    nc.tensor.matmul(ps[:, 0:HD],
                                     xq[:, ct, kt * 128:(kt + 1) * 128],
                                     wv_sb[:, ct, :],
                                     start=(ct == 0), stop=(ct == CT - 1))
                nc.vector.tensor_copy(
                    v_aug[:, kt, :, 0:D],
                    ps[:, 0:HD].rearrange("p (h d) -> p h d", h=HPG))

            warm = consts.tile([128, 512], BF16, tag="warm")
            nc.vector.memset(warm[:], 0.0)
            for _ in range(8):
                wps = jps.tile([128, 512], F32, tag="pj")
                nc.tensor.matmul(wps[:, :], warm[:, 0:128], warm[:, :],
                                 start=True, stop=True)

            blocks = [(s, min(512, kp - s)) for s in range(0, kp, 512)]

            # ---- attention + projection, span-major ----------------------
            span_on = []
            for (qs, qw) in spans:
                on_pair = onsb.tile([128, qw], BF16, tag="on_pair")
                on_solo = onsb.tile([65, qw], BF16, tag="on_solo")
                nc.vector.memset(on_solo[64:65, :qw], 1.0)    # bias row
                span_on.append((on_pair, on_solo))

            def head_src(h):
                if h < 2:
                    return k_pair, 64 * h, q_pair, 64 * h
                return kq_pack, 0, q_solo, 0

            def emit_s_pair(qs, qw, h, kt2):
                """Scores for key tiles kt2, kt2+1 into one 2-bank psum tile
                (one key tile per bank), so a single exp covers both."""
                k_src, k_lo, q_src, q_lo = head_src(h)
                n = min(2, kt_n - kt2)
                s_ps = aps.tile([128, 2, 512], F32, tag="s")
                for j in range(n):
                    kt = kt2 + j
                    nc.tensor.matmul(
                        s_ps[:, j, :qw],
                        k_src[k_lo:k_lo + D, kt * 128:(kt + 1) * 128],
                        q_src[q_lo:q_lo + D, qs:qs + qw],
                        start=True, stop=True)
                return s_ps, n

            def emit_exp(qw, s_ps, n):
                attn = asb.tile([128, 2, qw], BF16, tag="attn")
                nc.scalar.activation(attn[:, 0:n, :qw], s_ps[:, 0:n, :qw],
                                     mybir.ActivationFunctionType.Exp,
                                     scale=SCALE)
                return attn

            def emit_o(qs, qw, h, attns, on_pair, on_solo):
                """Accumulate attn @ v_aug over all key tiles, then normalize
                straight out of psum: out * bcast(1/sums), broadcast on the
                (otherwise idle) gpsimd engine."""
                o_ps = ops.tile([D + 1, qw], F32, tag="o")
                for (attn, kt2, n) in attns:
                    for j in range(n):
                        kt = kt2 + j
                        nc.tensor.matmul(o_ps[:, :qw], v_aug[:, kt, h, :],
                                         attn[:, j, :qw],
                                         start=(kt == 0),
                                         stop=(kt == kt_n - 1))
                rec = nsb.tile([1, qw], F32, tag="rec")
                nc.vector.reciprocal(rec[:, :qw], o_ps[D:D + 1, :qw])
                rec_bc = nsb.tile([D, qw], F32, tag="rec_bc")
                nc.gpsimd.partition_broadcast(rec_bc[:, :qw], rec[:, :qw])
                dst, dlo = (on_pair, 64 * h) if h < 2 else (on_solo, 0)
                nc.vector.tensor_mul(dst[dlo:dlo + D, :qw],
                                     o_ps[0:D, :qw], rec_bc[:, :qw])

            def emit_attn_head(qs, qw, h, on_pair, on_solo):
                """One (span, head): software-pipelined so the PE queue holds
                the next score pair before o(pair) and never stalls on exp."""
                o_ps = ops.tile([D + 1, qw], F32, tag="o")

                def exp_and_o(s_ps, n, kt2):
                    attn = emit_exp(qw, s_ps, n)
                    for j in range(n):
                        kt = kt2 + j
                        nc.tensor.matmul(o_ps[:, :qw], v_aug[:, kt, h, :],
                                         attn[:, j, :qw],
                                         start=(kt == 0),
                                         stop=(kt == kt_n - 1))

                prev = None
                for kt2 in range(0, kt_n, 2):
                    s_ps, n = emit_s_pair(qs, qw, h, kt2)
                    if prev is not None:
                        exp_and_o(*prev)
                    prev = (s_ps, n, kt2)
                exp_and_o(*prev)

                rec = nsb.tile([1, qw], F32, tag="rec")
                nc.vector.reciprocal(rec[:, :qw], o_ps[D:D + 1, :qw])
                rec_bc = nsb.tile([D, qw], F32, tag="rec_bc")
                nc.gpsimd.partition_broadcast(rec_bc[:, :qw], rec[:, :qw])
                dst, dlo = (on_pair, 64 * h) if h < 2 else (on_solo, 0)
                nc.vector.tensor_mul(dst[dlo:dlo + D, :qw],
                                     o_ps[0:D, :qw], rec_bc[:, :qw])

            def emit_proj(qs, qw, on_pair, on_solo, ci, mix=True,
                          part=None):
                """Output projection for one span; bias rides along as the
                65th contraction row of the solo matmul.  `part=(i, n)` emits
                only the i-th of n interleaved chunks of the qc loop."""
                qcs = list(range(0, qw, 128))
                if part is not None:
                    qcs = qcs[part[0]::part[1]]
                for qc in qcs:
                    cw = min(128, qw - qc)
                    for cs in range(0, C, 512):
                        ccw = min(512, C - cs)
                        pj = jps.tile([128, 512], F32, tag="pj")
                        nc.tensor.matmul(pj[:cw, :ccw],
                                         on_pair[:, qc:qc + cw],
                                         wo_pair[:, cs:cs + ccw],
                                         start=True, stop=False)
                        nc.tensor.matmul(pj[:cw, :ccw],
                                         on_solo[:, qc:qc + cw],
                                         wo_solo[:, cs:cs + ccw],
                                         start=False, stop=True)
                        pj_sb = jsb.tile([128, 512], BF16, tag="pj_sb")
                        if mix and ci % 2 == 0:
                            nc.scalar.copy(pj_sb[:cw, :ccw], pj[:cw, :ccw])
                        else:
                            nc.vector.tensor_copy(pj_sb[:cw, :ccw],
                                                  pj[:cw, :ccw])
                        ci += 1
                        nc.sync.dma_start(
                            rs_in[qs + qc:qs + qc + cw, cs:cs + ccw],
                            pj_sb[:cw, :ccw])
                return ci

            # Early phase: s+exp for heads 0/1 of the first two spans,
            # block-interleaved with QKV so the first exps start as soon as
            # the first k/q block lands, and the Act engine saturates while
            # the PE grinds through QKV.  o-accumulation is deferred until
            # v lands.
            early = [(si, h) for si in range(min(2, len(spans)))
                     for h in range(2)]
            npairs = -(-kt_n // 2)
            attns = {(si, h): [None] * npairs for (si, h) in early}

            def emit_unit(si, h, p):
                qs, qw = spans[si]
                kt2 = 2 * p
                s_ps, n = emit_s_pair(qs, qw, h, kt2)
                attns[si, h][p] = (emit_exp(qw, s_ps, n), kt2, n)

            # span-1 units keyed to k-block arrival
            for bi, (s, w) in enumerate(blocks):
                emit_w_block(wk_sb, k_pair, s, w)
                emit_w_block(wq_sb, q_pair, s, w, qlim=cnt)
                for p in range(npairs):
                    need = -(-min(2 * p + 2, kt_n) * 128 // 512) - 1
                    if need == bi:
                        for (si, h) in early[:2]:
                            emit_unit(si, h, p)

            # solo k/q group right away (span-1 head 2 needs it soon);
            # q_solo is un-packed per block so each small DMA only waits on
            # its own block's copy and never wedges the SP queue
            for (s, w) in blocks:
                emit_w_block(wkq_sb, kq_pack, s, w)
                nc.sync.dma_start(q_solo[:, s:s + w],
                                  kq_pack[64:128, s:s + w])

            def emit_vmask(h):          # sums column: 1.0 iff real key
                nc.vector.tensor_copy(v_aug[:, :, h, D:D + 1],
                                      vmk_sb.rearrange("p (t o) -> p t o",
                                                       o=1))

            # span-2 units woven with the remaining QKV work
            fillers = [(emit_vmask, (h,)) for h in range(HPG)]
            for kt in range(kt_n):
                fillers.append((emit_v, (kt,)))

            for (si, h) in early[2:]:
                for p in range(npairs):
                    emit_unit(si, h, p)
                    if fillers:
                        fn, args = fillers.pop(0)
                        fn(*args)
                    if fillers:
                        fn, args = fillers.pop(0)
                        fn(*args)
            for fn, args in fillers:
                fn(*args)

            def emit_units(si, h):
                attns[si, h] = []
                for p in range(npairs):
                    qs, qw = spans[si]
                    s_ps, n = emit_s_pair(qs, qw, h, 2 * p)
                    attns[si, h].append((emit_exp(qw, s_ps, n), 2 * p, n))

            ci = 0
            if len(spans) == 1:
                emit_units(0, 2)
                emit_o(*spans[0], 0, attns[0, 0], *span_on[0])
                emit_o(*spans[0], 1, attns[0, 1], *span_on[0])
                emit_o(*spans[0], 2, attns[0, 2], *span_on[0])
                ci = emit_proj(*spans[0], *span_on[0], ci)
            else:
                emit_units(0, 2)
                emit_o(*spans[0], 0, attns[0, 0], *span_on[0])
                emit_o(*spans[0], 1, attns[0, 1], *span_on[0])
                emit_o(*spans[0], 2, attns[0, 2], *span_on[0])
                emit_units(1, 2)
                emit_o(*spans[1], 0, attns[1, 0], *span_on[1])
                emit_o(*spans[1], 1, attns[1, 1], *span_on[1])
                ci = emit_proj(*spans[0], *span_on[0], ci, mix=False)
                if len(spans) > 2:
                    emit_o(*spans[1], 2, attns[1, 2], *span_on[1])
                for si in range(2, len(spans)):
                    for h in range(HPG):
                        emit_units(si, h)
                    emit_o(*spans[si], 0, attns[si, 0], *span_on[si])
                    ci = emit_proj(*spans[si - 1], *span_on[si - 1], ci)
                    emit_o(*spans[si], 1, attns[si, 1], *span_on[si])
                # final span: pair-contraction proj matmuls start before the
                # solo head's o/normalize so only the solo matmul + copy +
                # DMA remain on the end-of-kernel chain
                qs, qw = spans[-1]
                on_pair, on_solo = span_on[-1]
                if qw <= 128 and len(spans) > 1:
                    pjs = []
                    for cs in range(0, C, 512):
                        ccw = min(512, C - cs)
                        pj = jps.tile([128, 512], F32, tag="pj")
                        nc.tensor.matmul(pj[:qw, :ccw], on_pair[:, 0:qw],
                                         wo_pair[:, cs:cs + ccw],
                                         start=True, stop=False)
                        pjs.append((cs, ccw, pj))
                    emit_o(qs, qw, 2, attns[len(spans) - 1, 2],
                           on_pair, on_solo)
                    for (cs, ccw, pj) in pjs:
                        nc.tensor.matmul(pj[:qw, :ccw], on_solo[:, 0:qw],
                                         wo_solo[:, cs:cs + ccw],
                                         start=False, stop=True)
                        pj_sb = jsb.tile([128, 512], BF16, tag="pj_sb")
                        nc.vector.tensor_copy(pj_sb[:qw, :ccw], pj[:qw, :ccw])
                        nc.sync.dma_start(rs_in[qs:qs + qw, cs:cs + ccw],
                                          pj_sb[:qw, :ccw])
                else:
                    emit_o(qs, qw, 2, attns[len(spans) - 1, 2],
                           on_pair, on_solo)
                    ci = emit_proj(qs, qw, on_pair, on_solo, ci)

        # ---- chunked reduce-scatter over the 4 cores of this batch -------
        if with_rs:
            o = 0
            for (r0, r1) in chunks:
                q = (r1 - r0) // 4
                nc.gpsimd.collective_compute(
                    "ReduceScatter", mybir.AluOpType.add,
                    replica_groups=[[0, 1, 2, 3], [4, 5, 6, 7]],
                    ins=[rs_in[r0:r1, :]],
                    outs=[rs_out[o:o + q, :]])
                nc.sync.dma_start(out_ext[o:o + q, :], rs_out[o:o + q, :])
                o += q
        else:
            nc.sync.dma_start(out_ext[:], rs_in[0:out_ext.shape[0], :])


def make_in_maps(inputs, kept, cnt, kp):
    x = np.asarray(inputs["x"], dtype=np.float32)
    Wq, Wk, Wv, Wo = (np.asarray(inputs[k], np.float32)
                      for k in ("Wq", "Wk", "Wv", "Wo"))
    bo = np.asarray(inputs["bo"], np.float32)
    woT_full = np.ascontiguousarray(Wo.T)          # [hd_in, c_out]
    in_maps = []
    for c in range(NCORES):
        b, g = divmod(c, NGROUPS)
        g0 = g * HD
        xc = np.zeros((kp, C), np.float32)
        xc[:cnt[b]] = x[b][kept[b]]
        vmk_flat = np.zeros(kp, np.float32)
        vmk_flat[:cnt[b]] = 1.0
        vmk = np.ascontiguousarray(vmk_flat.reshape(kp // 128, 128).T)
        wkq = np.concatenate([Wk[g0 + 128:g0 + HD],
                              Wq[g0 + 128:g0 + HD]], axis=0)
        woT = np.concatenate([woT_full[g0:g0 + HD],
                              (bo / NGROUPS).reshape(1, C)], axis=0)
        in_maps.append({
            "xcT": np.ascontiguousarray(xc.T).astype(NPBF16),
            "vmk": vmk.astype(NPBF16),
            "wqp": np.ascontiguousarray(Wq[g0:g0 + 128].T).astype(NPBF16),
            "wkp": np.ascontiguousarray(Wk[g0:g0 + 128].T).astype(NPBF16),
            "wkq": np.ascontiguousarray(wkq.T).astype(NPBF16),
            "wvT": np.ascontiguousarray(Wv[g0:g0 + HD].T).astype(NPBF16),
            "woT": np.ascontiguousarray(woT).astype(NPBF16),
        })
    return in_maps


def kernel(x, mask, Wq, Wk, Wv, Wo, bo):
    x = np.asarray(x, dtype=np.float32)
    mask = np.asarray(mask)
    Wq, Wk, Wv, Wo = (np.asarray(w, np.float32) for w in (Wq, Wk, Wv, Wo))
    bo = np.asarray(bo, np.float32)
    kept = [np.nonzero(mask[b])[0] for b in range(B)]
    cnt = [len(k) for k in kept]
    cnt_max = max(max(cnt), 1)
    kp = max(128, -(-cnt_max // 128) * 128)

    nc = _build(kp, cnt_max)
    in_maps = make_in_maps(
        {"x": x, "Wq": Wq, "Wk": Wk, "Wv": Wv, "Wo": Wo, "bo": bo},
        kept, cnt, kp)

    r = run_bass_kernel_spmd(nc, in_maps, core_ids=list(range(NCORES)))
    globals()["LAST_HW_NS"] = r.exec_time_ns or r.mean_exec_time_ns
    globals()["LAST_RESULT"] = r

    # masked rows: uniform attention over ALL rows -> host-computed row
    chunks = _rs_chunks(kp, cnt_max)
    out = np.empty((B, N, C), np.float32)
    for b in range(B):
        parts = [np.asarray(r.results[NGROUPS * b + i]["out"],
                            np.float32) for i in range(NGROUPS)]
        rows_full = np.empty((kp, C), np.float32)
        o = 0
        for (r0, r1) in chunks:
            q = (r1 - r0) // 4
            for i in range(NGROUPS):
                rows_full[r0 + i * q:r0 + (i + 1) * q] = parts[i][o:o + q]
            o += q
        out[b, kept[b]] = rows_full[:cnt[b]]
        mv = x[b].mean(0) @ Wv.T
        out[b, mask[b] == 0] = mv @ Wo.T + bo
    return out
